# revision 20
# baseline (speedup 1.0000x reference)
"""Grayscale + single-level 2x2 Haar DWT kernel for Trainium2 (8 cores, SPMD).

Full input x [16,3,1024,1024] f32 -> out [16,4,512,512] f32.
Batch-sharded: core i handles samples [2i, 2i+1].

Math per sample (BGR weights w=(0.114,0.587,0.299), all bands scaled by 0.5):
  gray = w0*x[0] + w1*x[1] + w2*x[2]
  a,b,c,d = gray[0::2,0::2], gray[0::2,1::2], gray[1::2,0::2], gray[1::2,1::2]
  cA,cH,cV,cD = 0.5*(a+b+c+d), 0.5*(a+b-c-d), 0.5*(a-b+c-d), 0.5*(a-b-c+d)

Layout: a "superband" is 512 consecutive input rows loaded as one 2MB
contiguous DMA into a [128, 4, 1024] tile (partition p holds rows
4p..4p+3). Two superbands cover one sample plane.

Engine split (per superband):
  ACT : c_i = x_i * (w_i/2) cast f32->fp16, for channels 0 and 1 only
        (scale folded into the ACTIVATE-Copy cast)
  DVE : t = c0+c1 (fp16 dense TT, 2x_1p)
        g = ch2_f32*(w2/2) + t (scalar_tensor_tensor, 1x - folds the
        third channel's scale+cast, keeping ACT off the tail path)
        srow/drow = g[:,0::2,:] +/- g[:,1::2,:] (fp16 dense, 2x;
        written into t's rows - t is dead after g)
        cA,cH,cV,cD = srow/drow[...,0::2] +/- [...,1::2] (f32 out, 1x)
  SYNC: all DMA descriptor gen (loads + stores) on the SP HWDGE ring;
        software-pipelined so loads for superband i+1 are issued before
        compute of superband i.

The LAST superband is processed as two half-pieces with dedicated
half-size tiles (no slot sharing with the other half -> no cross-engine
WAR serialization) and channel-interleaved 1MB loads, shrinking the
serial drain chain after the final load lands.
"""

import numpy as np

N_CORES = 8
B, C, H, W = 16, 3, 1024, 1024
HO, WO = H // 2, W // 2
SPC = B // N_CORES  # samples per core
SB = 2              # superbands per sample plane (512 input rows each)

W_BGR = (0.114, 0.587, 0.299)

_compiled = None


def _build():
    from concourse import bacc, mybir
    from concourse.tile import TileContext

    f32 = mybir.dt.float32
    f16 = mybir.dt.float16
    add = mybir.AluOpType.add
    sub = mybir.AluOpType.subtract
    mult = mybir.AluOpType.mult

    w0h, w1h, w2h = (w * 0.5 for w in W_BGR)

    nc = bacc.Bacc("TRN2", target_bir_lowering=False, debug=False)
    # same bytes as [SPC, C, H, W] f32, pre-shaped for superband DMA
    x = nc.declare_dram_parameter("x", [SPC, C, SB, 128, 4, W], f32, isOutput=False)
    out = nc.declare_dram_parameter(
        "out", [SPC, 4, SB, 128, 2, WO], f32, isOutput=True
    )

    chunks = [(s, sb) for s in range(SPC) for sb in range(SB)]
    last = len(chunks) - 1

    with TileContext(nc) as tc:
        with (
            tc.tile_pool(name="in_pool", bufs=2) as in_pool,
            tc.tile_pool(name="sc_pool", bufs=2) as sc_pool,
            tc.tile_pool(name="mid_pool", bufs=2) as mid_pool,
            tc.tile_pool(name="out_pool", bufs=2) as out_pool,
        ):
            in_tiles = {}

            def issue_loads(i):
                s, sb = chunks[i]
                ts = []
                for ch in range(C):
                    t = in_pool.tile(
                        [128, 4, W], f32, tag=f"in{ch}", name=f"in{ch}_{i}"
                    )
                    ts.append(t)
                if i == last or i == 0:
                    # channel-interleaved half loads so the first/final
                    # pieces complete (all 3 channels) as early as possible
                    for h in range(2):
                        for ch in range(C):
                            nc.sync.dma_start(
                                out=ts[ch][:, 2 * h : 2 * h + 2, :],
                                in_=x[s, ch, sb, :, 2 * h : 2 * h + 2, :],
                            )
                else:
                    for ch in range(C):
                        nc.sync.dma_start(out=ts[ch][:, :, :], in_=x[s, ch, sb])
                in_tiles[i] = ts

            def butterfly(i, s, sb, srow, drow, ge, go, o_tiles, o_sl, store):
                # stage 1: row combine (dense fp16, 2x)
                nc.vector.tensor_tensor(srow, ge, go, add)
                nc.vector.tensor_tensor(drow, ge, go, sub)
                # stage 2: column combine (strided, f32 out) + store
                for band, (src, op) in enumerate(
                    ((srow, add), (drow, add), (srow, sub), (drow, sub))
                ):
                    # band order: cA, cH, cV, cD
                    o = o_tiles[band]
                    nc.vector.tensor_tensor(
                        o[:, o_sl, :], src[:, :, 0:W:2], src[:, :, 1:W:2], op
                    )
                    if store:
                        # whole-tile 512KB contiguous store once per chunk;
                        # SWDGE (gpsimd Q7) ring: store desc-gen never blocks
                        # the load ring or the compute engines
                        nc.gpsimd.dma_start(out=out[s, band, sb], in_=o[:, :, :])

            def make_out_tiles(i):
                return [
                    out_pool.tile(
                        [128, 2, WO], f32, tag=f"o{band}", name=f"o{band}_{i}"
                    )
                    for band in range(4)
                ]

            def compute_full(i):
                s, sb = chunks[i]
                ch_t = in_tiles.pop(i)
                sc0 = sc_pool.tile([128, 4, W], f16, tag="sc0", name=f"sc0_{i}")
                sc1 = sc_pool.tile([128, 4, W], f16, tag="sc1", name=f"sc1_{i}")
                t = mid_pool.tile([128, 4, W], f16, tag="t", name=f"t_{i}")
                nc.scalar.mul(sc0[:, :, :], ch_t[0][:, :, :], w0h)
                nc.scalar.mul(sc1[:, :, :], ch_t[1][:, :, :], w1h)
                nc.vector.tensor_tensor(
                    t[:, :, :], sc0[:, :, :], sc1[:, :, :], add
                )
                g = sc0  # dead after the TT; reuse its slot
                nc.vector.scalar_tensor_tensor(
                    g[:, :, :], ch_t[2][:, :, :], w2h, t[:, :, :], mult, add
                )
                butterfly(
                    i, s, sb,
                    t[:, 0:2, :], t[:, 2:4, :],
                    g[:, 0:4:2, :], g[:, 1:4:2, :],
                    make_out_tiles(i), slice(0, 2), True,
                )

            def compute_half(i, h, o_tiles):
                s, sb = chunks[i]
                ch_t = in_tiles[i] if h == 0 else in_tiles.pop(i)
                hid = f"{i}_{h}"
                hsl = slice(2 * h, 2 * h + 2)
                sc0 = sc_pool.tile(
                    [128, 2, W], f16, tag="sch0", name=f"sch0_{hid}"
                )
                sc1 = sc_pool.tile(
                    [128, 2, W], f16, tag="sch1", name=f"sch1_{hid}"
                )
                t = mid_pool.tile([128, 2, W], f16, tag="th", name=f"th_{hid}")
                nc.scalar.mul(sc0[:, :, :], ch_t[0][:, hsl, :], w0h)
                nc.scalar.mul(sc1[:, :, :], ch_t[1][:, hsl, :], w1h)
                nc.vector.tensor_tensor(
                    t[:, :, :], sc0[:, :, :], sc1[:, :, :], add
                )
                g = sc0
                nc.vector.scalar_tensor_tensor(
                    g[:, :, :], ch_t[2][:, hsl, :], w2h, t[:, :, :], mult, add
                )
                butterfly(
                    i, s, sb,
                    t[:, 0:1, :], t[:, 1:2, :],
                    g[:, 0:1, :], g[:, 1:2, :],
                    o_tiles, slice(h, h + 1), h == 1,
                )

            def compute_and_store(i):
                if i == last or i == 0:
                    o_tiles = make_out_tiles(i)
                    compute_half(i, 0, o_tiles)
                    compute_half(i, 1, o_tiles)
                else:
                    compute_full(i)

            for i in range(len(chunks)):
                issue_loads(i)
                if i >= 1:
                    compute_and_store(i - 1)
            compute_and_store(last)
    nc.finalize()
    return nc


def kernel(x: np.ndarray) -> np.ndarray:
    global _compiled
    from concourse.bass_utils import run_bass_kernel_spmd

    if _compiled is None:
        _compiled = _build()
    nc = _compiled

    x = np.ascontiguousarray(x, dtype=np.float32)
    in_maps = [{"x": x[i * SPC : (i + 1) * SPC]} for i in range(N_CORES)]
    res = run_bass_kernel_spmd(nc, in_maps, list(range(N_CORES))).results
    out = np.concatenate(
        [r["out"].reshape(SPC, 4, HO, WO) for r in res], axis=0
    )
    return out


# revision 21
# speedup vs baseline: 1.1099x; 1.1099x over previous
"""Grayscale + single-level 2x2 Haar DWT kernel for Trainium2 (8 cores, SPMD).

Full input x [16,3,1024,1024] f32 -> out [16,4,512,512] f32.
Batch-sharded: core i handles samples [2i, 2i+1].

Math per sample (BGR weights w=(0.114,0.587,0.299), all bands scaled by 0.5):
  gray = w0*x[0] + w1*x[1] + w2*x[2]
  a,b,c,d = gray[0::2,0::2], gray[0::2,1::2], gray[1::2,0::2], gray[1::2,1::2]
  cA,cH,cV,cD = 0.5*(a+b+c+d), 0.5*(a+b-c-d), 0.5*(a-b+c-d), 0.5*(a-b-c+d)

Structure: uniform pieces of 256 consecutive input rows, loaded as one
contiguous 1MB DMA per channel into [128, 2, 1024] tiles (partition p
holds input rows 2p, 2p+1 of the piece -> output row p). 4 pieces per
sample plane, 8 per core. Outputs store as contiguous 256KB per band.

Engine split (per piece):
  ACT : c_i = x_i * (w_i/2) cast f32->fp16, channels 0/1 (ACTIVATE-Copy
        with folded scale)
  DVE : t = c0+c1 (fp16 dense TT, 2x_1p mode)
        g = ch2_f32*(w2/2) + t (scalar_tensor_tensor, 1x - folds the
        third channel's scale+cast, keeping ACT off the drain path)
        srow/drow = g[:,0,:] +/- g[:,1,:] (fp16 dense 2x, into t's rows)
        cA,cH,cV,cD = srow/drow[...,0::2] +/- [...,1::2] (f32 out, 1x)
  SYNC: load descriptor gen only (nothing ever blocks the load ring)
  GPSIMD/SWDGE: store descriptor gen (separate queue, never blocks
        loads or compute)

Pipelined with 4-deep input buffering; DMA is the bottleneck engine
(~33.5 MB/core at ~430 GB/s), DVE ~55-70% busy, ACT ~35% busy.
"""

import numpy as np

N_CORES = 8
B, C, H, W = 16, 3, 1024, 1024
HO, WO = H // 2, W // 2
SPC = B // N_CORES   # samples per core
PCS = H // 256       # pieces per sample plane (256 input rows each)

W_BGR = (0.114, 0.587, 0.299)

_compiled = None


def _build():
    from concourse import bacc, mybir
    from concourse.tile import TileContext

    f32 = mybir.dt.float32
    f16 = mybir.dt.float16
    add = mybir.AluOpType.add
    sub = mybir.AluOpType.subtract
    mult = mybir.AluOpType.mult

    w0h, w1h, w2h = (w * 0.5 for w in W_BGR)

    nc = bacc.Bacc("TRN2", target_bir_lowering=False, debug=False)
    # same bytes as [SPC, C, H, W] f32, pre-shaped for piece DMA
    x = nc.declare_dram_parameter(
        "x", [SPC, C, PCS, 128, 2, W], f32, isOutput=False
    )
    out = nc.declare_dram_parameter(
        "out", [SPC, 4, PCS, 128, 1, WO], f32, isOutput=True
    )

    pieces = [(s, p) for s in range(SPC) for p in range(PCS)]
    n = len(pieces)

    with TileContext(nc) as tc:
        with (
            tc.tile_pool(name="in_pool", bufs=4) as in_pool,
            tc.tile_pool(name="sc_pool", bufs=3) as sc_pool,
            tc.tile_pool(name="mid_pool", bufs=2) as mid_pool,
            tc.tile_pool(name="out_pool", bufs=2) as out_pool,
        ):
            in_tiles = {}

            def issue_loads(i):
                s, p = pieces[i]
                ts = []
                for ch in range(C):
                    t = in_pool.tile(
                        [128, 2, W], f32, tag=f"in{ch}", name=f"in{ch}_{i}"
                    )
                    nc.sync.dma_start(out=t[:, :, :], in_=x[s, ch, p])
                    ts.append(t)
                in_tiles[i] = ts

            def compute_and_store(i):
                s, p = pieces[i]
                ch_t = in_tiles.pop(i)
                sc0 = sc_pool.tile([128, 2, W], f16, tag="sc0", name=f"sc0_{i}")
                sc1 = sc_pool.tile([128, 2, W], f16, tag="sc1", name=f"sc1_{i}")
                t = mid_pool.tile([128, 2, W], f16, tag="t", name=f"t_{i}")
                nc.scalar.mul(sc0[:, :, :], ch_t[0][:, :, :], w0h)
                nc.scalar.mul(sc1[:, :, :], ch_t[1][:, :, :], w1h)
                nc.vector.tensor_tensor(
                    t[:, :, :], sc0[:, :, :], sc1[:, :, :], add
                )
                g = sc0  # dead after the TT; reuse its slot
                nc.vector.scalar_tensor_tensor(
                    g[:, :, :], ch_t[2][:, :, :], w2h, t[:, :, :], mult, add
                )
                # stage 1: row combine into t's rows (t is dead after g)
                srow, drow = t[:, 0:1, :], t[:, 1:2, :]
                nc.vector.tensor_tensor(srow, g[:, 0:1, :], g[:, 1:2, :], add)
                nc.vector.tensor_tensor(drow, g[:, 0:1, :], g[:, 1:2, :], sub)
                # stage 2: column combine (strided, f32 out) + store
                for band, (src, op) in enumerate(
                    ((srow, add), (drow, add), (srow, sub), (drow, sub))
                ):
                    # band order: cA, cH, cV, cD
                    o = out_pool.tile(
                        [128, 1, WO], f32, tag=f"o{band}", name=f"o{band}_{i}"
                    )
                    nc.vector.tensor_tensor(
                        o[:, :, :], src[:, :, 0:W:2], src[:, :, 1:W:2], op
                    )
                    # SWDGE (gpsimd Q7) ring: store desc-gen never blocks the
                    # load ring or the compute engines
                    nc.gpsimd.dma_start(out=out[s, band, p], in_=o[:, :, :])

            for i in range(n):
                issue_loads(i)
                if i >= 2:
                    compute_and_store(i - 2)
            compute_and_store(n - 2)
            compute_and_store(n - 1)
    nc.finalize()
    return nc


def kernel(x: np.ndarray) -> np.ndarray:
    global _compiled
    from concourse.bass_utils import run_bass_kernel_spmd

    if _compiled is None:
        _compiled = _build()
    nc = _compiled

    x = np.ascontiguousarray(x, dtype=np.float32)
    in_maps = [{"x": x[i * SPC : (i + 1) * SPC]} for i in range(N_CORES)]
    res = run_bass_kernel_spmd(nc, in_maps, list(range(N_CORES))).results
    out = np.concatenate(
        [r["out"].reshape(SPC, 4, HO, WO) for r in res], axis=0
    )
    return out
